# revision 27
# baseline (speedup 1.0000x reference)
"""Trainium2 Bass kernel for nn_AppearanceComposability (sparse_attention).

Reference semantics, per (b, c) with 64x64 images, 3x3 unfold (pad 1):
  out_flat[m] = K_flat[m] * qv[m // 9],   qv[i] = Q_flat[9*i + 4]
where K_flat / Q_flat are the per-channel flattened unfold blocks
(kk*4096 + l, channel order (C, kh, kw)).

v7 implementation (bf16 end-to-end; rel err ~2.9e-3 vs gate 2e-2;
measured 82.9us vs 83.2us prior best / 93-96us same-day baseline):
  - two compute engines only: ACT + DVE. GPSIMD is left idle on purpose:
    it arbitrates an EXCLUSIVE SBUF port pair with DVE, so any GPSIMD op
    fully blocks DVE tensor_tensor (measured 4x TT slowdowns) - GPSIMD
    compute is strictly negative alongside DVE TTs.
  - 13 chunks "stretched": ACT builds qs[l]=qv[(s+l)//9] by broadcast
    copy straight from the padded query image (own SBUF ports, truly
    parallel with DVE), DVE runs flat contiguous bf16 TTs at 2x packing
    (~0.56 ns/elem solo-clock); 5 chunks broadcast on DVE (1x) placed
    where ACT's production lag would otherwise idle DVE
  - broadcast chunks read a compact qv array: big segments built on ACT,
    tiny (<24 group) boundary segments and query-x-wrap zeros on DVE
    (ACT's ~280-cycle per-ACTIVATE overhead makes small ops expensive);
    same big/tiny split for the stretch segments -> single big TT per
    bcast chunk
  - per-chunk out tiles with 8-elem margins absorb the bcast group
    overhang (no cross-chunk ordering hazards)
  - loads: q0's first half split into quarters across BOTH HWDGE queues
    (first stretch starts ~3us earlier), k0's half second on scalar,
    the rest chained on sync; stores mostly on the sync queue (idle
    engine), 5 on scalar deferred into the ACT stream between ops, the
    last two split in halves across both queues for the tail drain
  - data parallel over batch: 8 cores, core b handles batch b
"""
import os
import sys

import numpy as np


def _ensure_path():
    try:
        import concourse  # noqa: F401
    except ImportError:
        for p in ("/opt/trn_rl_repo", "/root/.axon_site/_ro/trn_rl_repo"):
            if os.path.isdir(p):
                sys.path.insert(0, p)
                return


_ensure_path()

import concourse.bacc as bacc  # noqa: E402
import concourse.tile as tile  # noqa: E402
from concourse import mybir  # noqa: E402
from concourse.bass_utils import run_bass_kernel_spmd  # noqa: E402
from concourse.tile import add_dep_helper  # noqa: E402


def _install_ntff_hook_shim():
    """Provide antenv.axon_hooks when the image's antenv lacks it."""
    try:
        import antenv.axon_hooks  # noqa: F401
        return
    except ImportError:
        pass
    try:
        import types

        import antenv
        holder = {"hook": None, "tried": False}

        def set_axon_ntff_profile_hook(h):
            holder["hook"] = h
            holder["tried"] = True

        def get_axon_ntff_profile_hook():
            if not holder["tried"]:
                holder["tried"] = True
                try:
                    from trn_agent_boot.trn_boot import _ntff_profile_via_ctypes
                    so = "/opt/axon/libaxon_pjrt.so"
                    if os.path.exists(so):
                        holder["hook"] = _ntff_profile_via_ctypes(so)
                except Exception:
                    holder["hook"] = None
            return holder["hook"]

        mod = types.ModuleType("antenv.axon_hooks")
        mod.set_axon_ntff_profile_hook = set_axon_ntff_profile_hook
        mod.get_axon_ntff_profile_hook = get_axon_ntff_profile_hook
        sys.modules["antenv.axon_hooks"] = mod
        antenv.axon_hooks = mod
    except Exception:
        pass


_install_ntff_hook_shim()

F32 = mybir.dt.float32
BF16 = mybir.dt.bfloat16

B = 8          # batch == number of cores
C = 256        # channels
H = W = 64
L = H * W      # 4096 pixels
K2 = 9         # 3x3 patch
M = L * K2     # 36864 per-channel output length
MARG = 80      # input image margin (>= 73 needed)
OM = 8         # output tile margin (>= 8 needed)
QM = 8         # qs tile head margin
QTAIL = 580    # qs tile tail pad so run-zero rearrange views stay in-bounds
OFFS = [(kh - 1) * W + (kw - 1) for kh in range(3) for kw in range(3)]


def _ceil_div(a, b):
    return -(-a // b)


def _plan_qv_ops():
    """Per kk: (i_lo, i_hi, src_start, memsets) for qv[i] = Q_flat[9i+4].

    src position (relative to q image start at MARG) of qv[i] is
    src_start + 9*(i - i_lo).  memsets are (first, cnt, 64) runs in
    i-space where the query tap wraps an x-edge (must read as zero).
    """
    ops = []
    for kk in range(K2):
        s = L * kk
        i_lo = max(0, _ceil_div(s - 4, 9))
        i_hi = min(L, _ceil_div(s + L - 4, 9))
        src_start = 9 * i_lo + 4 - s + OFFS[kk]
        memsets = []
        kw = kk % 3
        if kw != 1:
            target = 0 if kw == 0 else 63
            i0 = (57 * (target - 4 + s)) % 64  # 57 = 9^-1 mod 64
            first = i_lo + ((i0 - i_lo) % 64)
            if first < i_hi:
                cnt = (i_hi - 1 - first) // 64 + 1
                memsets.append((first, cnt, 64))
        ops.append((i_lo, i_hi, src_start, memsets))
    return ops


def _plan_tt_ops():
    """Per kk: (g_lo, g_hi, ngroups, q0); TT covers l in [g_lo, g_hi)."""
    ops = []
    for kk in range(K2):
        s = L * kk
        g_lo = -(s % 9)
        g_hi = L + ((-(s + L)) % 9)
        ops.append((g_lo, g_hi, (g_hi - g_lo) // 9, (s + g_lo) // 9))
    return ops


QV_OPS = _plan_qv_ops()
TT_OPS = _plan_tt_ops()

# Per-(group, chunk) compute mode: 's' = stretched (ACT qs + DVE flat
# 2x TT), 'b' = broadcast TT on DVE (1x, compact qv operand).
BCAST = frozenset({(0, 0), (0, 7), (1, 0), (1, 4), (1, 7)})

# DVE master schedule: bcast chunks are placed where ACT's stretch
# production would otherwise leave DVE idle.
DVE_STEPS = [(0, 0), (0, 1), (0, 2), (0, 3), (0, 7), (0, 4), (0, 5),
             (0, 6), (0, 8), (1, 1), (1, 4), (1, 2), (1, 3), (1, 0),
             (1, 5), (1, 6), (1, 7), (1, 8)]

# Store queue per (g, kk): 'y' = sync, 'c' = scalar (deferred issue),
# '2' = split halves across both queues (tail drain).
STORE_Q = {
    (0, 0): 'c', (0, 1): 'y', (0, 2): 'c', (0, 3): 'y', (0, 4): 'y',
    (0, 5): 'c', (0, 6): 'y', (0, 7): 'y', (0, 8): 'y',
    (1, 0): 'y', (1, 1): 'c', (1, 2): 'y', (1, 3): 'y', (1, 4): 'y',
    (1, 5): 'c', (1, 6): 'y', (1, 7): '2', (1, 8): '2',
}

# Segments smaller than this many qv-groups go to DVE as tiny
# tensor_copies instead of paying ACT's ~280-cycle ACTIVATE overhead.
TINY = 24

# Head splits (group 0 only) so early pieces only need first half-loads.
B_HEAD_CUT = 230     # (0,0): qv groups [0, cut) need only q0h1/k0h1
ACT_HEAD_CUT = 685   # (0,1): qv groups [455, cut) need only q0h1


def _chunk_cover(kk):
    """Full qv-group coverage [i0, i1) of chunk kk's TT range."""
    glo, ghi, ng, q0 = TT_OPS[kk]
    i0 = (kk * L + glo) // 9
    return i0, i0 + ng


def build_graph():
    nc = bacc.Bacc(None, target_bir_lowering=False)
    key_ext = nc.declare_dram_parameter("key_map", [C, L], BF16,
                                        isOutput=False)
    query_ext = nc.declare_dram_parameter("query_map", [C, L], BF16,
                                          isOutput=False)
    out_ext = nc.declare_dram_parameter("out", [C, M], BF16, isOutput=True)

    ngroups = C // 128
    with tile.TileContext(nc) as tc:
        with (
            tc.tile_pool(name="pads", bufs=1) as pads,
            tc.tile_pool(name="qvp", bufs=1) as qvp,
            tc.tile_pool(name="qsp", bufs=3) as qsp,
            tc.tile_pool(name="outs", bufs=14) as outs,
        ):
            key_pads, q_pads, qvs = [], [], []
            for g in range(ngroups):
                q_pad = pads.tile([128, MARG + L + MARG], BF16,
                                  name=f"q_pad{g}", tag=f"q_pad{g}")
                nc.vector.memset(q_pad[:, 0:MARG], 0.0)
                nc.vector.memset(q_pad[:, MARG + L:MARG + L + MARG], 0.0)
                key_pad = pads.tile([128, MARG + L + MARG], BF16,
                                    name=f"key_pad{g}", tag=f"key_pad{g}")
                nc.vector.memset(key_pad[:, 0:MARG], 0.0)
                nc.vector.memset(key_pad[:, MARG + L:MARG + L + MARG], 0.0)
                key_pads.append(key_pad)
                q_pads.append(q_pad)
                qvs.append(qvp.tile([128, L], BF16,
                                    name=f"qv{g}", tag=f"qv{g}"))

            # Loads. The first q0 half is split into quarters across
            # BOTH queues so the first stretch (which needs all of q0's
            # first half) can start ~3us earlier; k0's first half rides
            # second on the scalar queue; the rest chained on sync.
            hL = L // 2
            qL = L // 4
            nc.scalar.dma_start(q_pads[0][:, MARG + qL:MARG + hL],
                                query_ext[0:128, qL:hL])
            nc.scalar.dma_start(key_pads[0][:, MARG:MARG + hL],
                                key_ext[0:128, 0:hL])
            prev_q = nc.sync.dma_start(q_pads[0][:, MARG:MARG + qL],
                                       query_ext[0:128, 0:qL])
            seq = [(0, "q", 1), (0, "k", 1),
                   (1, "q", 0), (1, "q", 1), (1, "k", 0), (1, "k", 1)]
            for (g, t, h) in seq:
                pad = q_pads[g] if t == "q" else key_pads[g]
                ext = query_ext if t == "q" else key_ext
                qd = nc.sync.dma_start(
                    pad[:, MARG + h * hL:MARG + (h + 1) * hL],
                    ext[g * 128:(g + 1) * 128, h * hL:(h + 1) * hL])
                add_dep_helper(qd.ins, prev_q.ins, sync=True,
                               reason="chain loads on sync queue")
                prev_q = qd

            # ---------- emission helpers ----------
            state = {"act": None, "dve": None}

            def chain(engine_key, op):
                prev = state[engine_key]
                if prev is not None:
                    add_dep_helper(op.ins, prev.ins, sync=False,
                                   reason=f"{engine_key} stream order")
                state[engine_key] = op
                return op

            def emit_qv(g, a, b):
                """Build qv[i] for i in [a, b) from q_pad: big segments
                on ACT, tiny segments on DVE, query-x-wrap zeros as DVE
                memsets (consumer-side engine, cheap sync)."""
                for kk2 in range(K2):
                    i_lo, i_hi, src_start, msets = QV_OPS[kk2]
                    c, dd = max(i_lo, a), min(i_hi, b)
                    if c >= dd:
                        continue
                    sa = MARG + src_start + 9 * (c - i_lo)
                    dst = qvs[g][:, c:dd]
                    src = q_pads[g][:, sa:sa + 9 * (dd - c):9]
                    if dd - c < TINY:
                        chain("dve", nc.vector.tensor_copy(dst, src))
                    else:
                        chain("act", nc.scalar.copy(dst, src))
                    for (first, cnt, step) in msets:
                        j0 = max(0, _ceil_div(c - first, step))
                        j1 = (dd - 1 - first) // step
                        if j0 > j1:
                            continue
                        f2 = first + j0 * step
                        c2 = j1 - j0 + 1
                        chain("dve", nc.vector.memset(
                            qvs[g][:, f2:f2 + (c2 - 1) * step + 1:step],
                            0.0))

            def emit_stretch(g, qs, s, a, b):
                """Build qs[QM + 9i - s .. +9] = qv[i] for qv-groups i
                in [a, b), reading straight out of q_pad. Big segments
                on ACT; tiny boundary segments on DVE."""
                for kk2 in range(K2):
                    i_lo, i_hi, src_start, _ = QV_OPS[kk2]
                    c, dd = max(i_lo, a), min(i_hi, b)
                    if c >= dd:
                        continue
                    dst = qs[:, QM + 9 * c - s:
                             QM + 9 * dd - s].rearrange(
                        "p (n k) -> p n k", k=9)
                    sa = MARG + src_start + 9 * (c - i_lo)
                    src = q_pads[g][:, sa:sa + 9 * (dd - c):9].unsqueeze(
                        2).broadcast_to([128, dd - c, 9])
                    if dd - c < TINY:
                        chain("dve", nc.vector.tensor_copy(dst, src))
                    else:
                        chain("act", nc.scalar.copy(dst, src))

            def emit_qs_runzeros(g, qs, s, a, b):
                """DVE: zero 9-wide qs runs where the query tap wrapped
                an x-edge (must precede the flat TT)."""
                for kk2 in range(K2):
                    i_lo, i_hi, _, msets = QV_OPS[kk2]
                    c, dd = max(i_lo, a), min(i_hi, b)
                    if c >= dd:
                        continue
                    for (first, cnt, step) in msets:
                        j0 = max(0, _ceil_div(c - first, step))
                        j1 = (dd - 1 - first) // step
                        if j0 > j1:
                            continue
                        iz = first + j0 * step
                        cnt2 = j1 - j0 + 1
                        A = QM + 9 * iz - s
                        view = qs[:, A:A + 576 * cnt2].rearrange(
                            "p (n k) -> p n k", k=576)[:, :, 0:9]
                        chain("dve", nc.vector.memset(view, 0.0))

            def emit_bcast(g, kk, ot, a, b):
                """DVE: one broadcast TT for qv-groups [a, b) of chunk
                (g, kk), contiguous qv operand."""
                s = kk * L
                n = b - a
                dst = ot[:, OM + 9 * a - s:
                         OM + 9 * b - s].rearrange("p (n k) -> p n k", k=9)
                src_k = key_pads[g][:, MARG + 9 * a - s + OFFS[kk]:
                                    MARG + 9 * b - s + OFFS[kk]].rearrange(
                    "p (n k) -> p n k", k=9)
                src_q = qvs[g][:, a:b].unsqueeze(2).broadcast_to(
                    [128, n, 9])
                chain("dve", nc.vector.tensor_mul(dst, src_k, src_q))

            def emit_colmset(kk, ot):
                """DVE: key-x-wrap zeros, stride-64 columns."""
                kw = kk % 3
                if kw == 1:
                    return
                c0 = 0 if kw == 0 else 63
                chain("dve", nc.vector.memset(ot[:, OM + c0:OM + L:64],
                                              0.0))

            # ---------- tiles / stores ----------
            ots = {}

            def get_ot(g, kk):
                if (g, kk) not in ots:
                    ots[(g, kk)] = outs.tile(
                        [128, OM + L + OM], BF16,
                        name=f"ot{g}_{kk}", tag="ot")
                return ots[(g, kk)]

            pending_scalar = []

            def flush_scalar():
                while pending_scalar:
                    nc.scalar.dma_start(*pending_scalar.pop(0))

            def emit_store(g, kk):
                ot = ots[(g, kk)]
                rows = slice(g * 128, (g + 1) * 128)
                q = STORE_Q[(g, kk)]
                c0 = kk * L
                if q == 'y':
                    nc.sync.dma_start(out_ext[rows, c0:c0 + L],
                                      ot[:, OM:OM + L])
                elif q == 'c':
                    pending_scalar.append(
                        (out_ext[rows, c0:c0 + L], ot[:, OM:OM + L]))
                else:  # '2': split halves across both queues
                    nc.sync.dma_start(out_ext[rows, c0:c0 + hL],
                                      ot[:, OM:OM + hL])
                    nc.scalar.dma_start(out_ext[rows, c0 + hL:c0 + L],
                                        ot[:, OM + hL:OM + L])

            # ---------- ACT program ----------
            qs_tiles = {}

            def _cv(kk):
                return _chunk_cover(kk)

            ACT_PROGRAM = [
                ("sA", 0, 1),                                 # 0
                ("qv", 0, 0, _cv(0)[0], B_HEAD_CUT),          # 1
                ("qv", 0, 0, B_HEAD_CUT, _cv(0)[1]),          # 2
                ("sB", 0, 1),                                 # 3
                ("s", 0, 2),                                  # 4
                ("qv", 0, 7, _cv(7)[0], _cv(7)[1]),           # 5
                ("s", 0, 3),                                  # 6
                ("s", 0, 4),                                  # 7
                ("s", 0, 5),                                  # 8
                ("s", 0, 6),                                  # 9
                ("s", 0, 8),                                  # 10
                ("qv", 1, 4, _cv(4)[0], _cv(4)[1]),           # 11
                ("s", 1, 1),                                  # 12
                ("s", 1, 2),                                  # 13
                ("qv", 1, 0, _cv(0)[0], _cv(0)[1]),           # 14
                ("s", 1, 3),                                  # 15
                ("s", 1, 5),                                  # 16
                ("s", 1, 6),                                  # 17
                ("qv", 1, 7, _cv(7)[0], _cv(7)[1]),           # 18
                ("s", 1, 8),                                  # 19
            ]
            WATERMARK = {
                (0, 0): 2, (0, 1): 4, (0, 2): 6, (0, 3): 7,
                (0, 7): 7, (0, 4): 8, (0, 5): 9, (0, 6): 10,
                (0, 8): 12, (1, 1): 13, (1, 4): 13, (1, 2): 15,
                (1, 3): 16, (1, 0): 16, (1, 5): 17, (1, 6): 19,
                (1, 7): 19, (1, 8): 19,
            }
            act_state = {"next": 0}

            def ensure_act(idx):
                while act_state["next"] <= idx:
                    ent = ACT_PROGRAM[act_state["next"]]
                    if ent[0] == "qv":
                        _, g, kk, a, b = ent
                        emit_qv(g, a, b)
                    else:
                        tag, g, kk = ent
                        s = kk * L
                        if (g, kk) not in qs_tiles:
                            qs_tiles[(g, kk)] = qsp.tile(
                                [128, QM + L + QM + QTAIL], BF16,
                                name=f"qs{g}_{kk}", tag="qs")
                        qs = qs_tiles[(g, kk)]
                        i0 = s // 9
                        i1 = _ceil_div(s + L, 9)
                        if tag == "sA":
                            emit_stretch(g, qs, s, i0, ACT_HEAD_CUT)
                        elif tag == "sB":
                            emit_stretch(g, qs, s, ACT_HEAD_CUT, i1)
                        else:
                            emit_stretch(g, qs, s, i0, i1)
                    act_state["next"] += 1
                    flush_scalar()

            # ---------- DVE master loop ----------
            for step in DVE_STEPS:
                g, kk = step
                ot = get_ot(g, kk)
                ensure_act(WATERMARK[step])
                flush_scalar()
                if step in BCAST:
                    i0, i1 = _chunk_cover(kk)
                    if step == (0, 0):
                        emit_bcast(g, kk, ot, i0, B_HEAD_CUT)
                        emit_bcast(g, kk, ot, B_HEAD_CUT, i1)
                    else:
                        emit_bcast(g, kk, ot, i0, i1)
                else:
                    s = kk * L
                    qs = qs_tiles[step]
                    i0 = s // 9
                    i1 = _ceil_div(s + L, 9)
                    key_pad = key_pads[g]
                    if step == (0, 1):
                        cut = 9 * ACT_HEAD_CUT - s - 4
                        cut -= cut % 2
                        emit_qs_runzeros(g, qs, s, i0, ACT_HEAD_CUT)
                        chain("dve", nc.vector.tensor_mul(
                            ot[:, OM:OM + cut],
                            key_pad[:, MARG + OFFS[kk]:
                                    MARG + OFFS[kk] + cut],
                            qs[:, QM:QM + cut]))
                        emit_qs_runzeros(g, qs, s, ACT_HEAD_CUT, i1)
                        chain("dve", nc.vector.tensor_mul(
                            ot[:, OM + cut:OM + L],
                            key_pad[:, MARG + OFFS[kk] + cut:
                                    MARG + OFFS[kk] + L],
                            qs[:, QM + cut:QM + L]))
                    else:
                        emit_qs_runzeros(g, qs, s, i0, i1)
                        chain("dve", nc.vector.tensor_mul(
                            ot[:, OM:OM + L],
                            key_pad[:, MARG + OFFS[kk]:
                                    MARG + OFFS[kk] + L],
                            qs[:, QM:QM + L]))
                emit_colmset(kk, ot)
                emit_store(g, kk)
            flush_scalar()
    nc.compile()
    return nc


_GRAPH_CACHE = {}


def _get_graph():
    if "nc" not in _GRAPH_CACHE:
        _GRAPH_CACHE["nc"] = build_graph()
    return _GRAPH_CACHE["nc"]


def kernel(key_map: np.ndarray, query_map: np.ndarray,
           _trace: bool = False, _tmpdir: str | None = None):
    import ml_dtypes
    bf16 = ml_dtypes.bfloat16
    key_map = np.ascontiguousarray(key_map, dtype=np.float32).astype(bf16)
    query_map = np.ascontiguousarray(query_map, dtype=np.float32).astype(bf16)
    assert key_map.shape == (B, C, H, W), key_map.shape

    nc = _get_graph()
    in_maps = [
        {"key_map": key_map[b].reshape(C, L),
         "query_map": query_map[b].reshape(C, L)}
        for b in range(B)
    ]
    res = run_bass_kernel_spmd(
        nc, in_maps, core_ids=list(range(B)),
        trace=_trace, tmpdir=_tmpdir,
    )
    out = np.stack([res.results[b]["out"] for b in range(B)])
    _GRAPH_CACHE["last_exec_time_ns"] = res.exec_time_ns
    _GRAPH_CACHE["last_results"] = res
    return out.astype(np.float32).reshape(B, C, L, K2)
